# revision 18
# baseline (speedup 1.0000x reference)
"""Chunked-causal attention with sinks on 8 TRN2 NeuronCores.

Sharding: the 64 (batch, head) pairs are split 8-per-core (data parallel on
B, tensor parallel on H). Each core runs the same Bass program over its 8
pairs x 4 chunks of 1024 tokens.

The per-core shard layout is chosen for DMA/TensorE efficiency:
  - Q, K arrive pre-transposed as bf16 [pairs, D, S]: the score matmul
    contracts over D, which must sit on SBUF partitions, and bf16 is the
    matmul compute dtype either way (the host conversion is numerically
    identical to an on-device cast). Per-partition rows are contiguous.
  - V arrives as bf16 [pairs, P, nch, T, D+1] (s = t*P + p within a chunk),
    with a ones column appended: partition-major so each partition's slice
    is one contiguous DRAM run, and the ones column makes the PV matmul
    emit the softmax denominator as output column D.
  - The output is stored partition-major [pairs, nch, P, T, D] and
    un-permuted on the host.

Per (pair, chunk) the kernel computes, entirely on-chip:
  S_T[k, q] = K @ Q^T          (TensorE, bf16; scores transposed so that the
                                PV matmul can consume exp(S_T) directly)
  P_T       = exp(S_T / sqrt(D))  (ScalarE; no max-subtraction -- softmax is
                                shift-invariant and scores here are O(5), so
                                fp32 exp is exact enough; masked entries are
                                zeroed after the exp)
  O[q, :]   = P_T^T @ [V | 1]  (TensorE; the ones column yields the softmax
                                denominator in column D of the same matmul)
  out       = O[:, :D] / (O[:, D] + exp(sink))
"""

import ml_dtypes
import numpy as np

import concourse.bacc as bacc
import concourse.bass as bass
import concourse.mybir as mybir
import concourse.tile as tile
from concourse.bass_utils import run_bass_kernel_spmd

N_CORES = 8
B, S, H, D = 4, 4096, 16, 128
C = 1024                # chunk size
NCH = S // C            # chunks per sequence
PAIRS = B * H           # 64 (batch, head) pairs
PPC = PAIRS // N_CORES  # pairs per core
P = 128                 # SBUF partitions
T = C // P              # 128-row tiles per chunk
SCALE = 1.0 / float(np.sqrt(D))

F32 = mybir.dt.float32
BF16 = mybir.dt.bfloat16


def _build_program(ppc=PPC, nch=NCH):
    s_len = nch * C
    nc = bacc.Bacc("TRN2", target_bir_lowering=False, debug=False)
    qt_d = nc.dram_tensor("qt", [ppc, D, s_len], BF16, kind="ExternalInput")
    kt_d = nc.dram_tensor("kt", [ppc, D, s_len], BF16, kind="ExternalInput")
    v_d = nc.dram_tensor("v", [ppc, P, nch, T, D + 1], BF16, kind="ExternalInput")
    es_d = nc.dram_tensor("esink", [ppc, P], F32, kind="ExternalInput")
    out_d = nc.dram_tensor("out", [ppc, P, nch, T, D], F32, kind="ExternalOutput")

    with tile.TileContext(nc) as tc:
        with (
            tc.tile_pool(name="loads", bufs=2) as loads,
            tc.tile_pool(name="ptile", bufs=2) as ppool,
            tc.tile_pool(name="outs", bufs=2) as opool,
            tc.tile_pool(name="small", bufs=4) as small,
            tc.tile_pool(name="spsum", bufs=2, space="PSUM") as spsum,
            tc.tile_pool(name="opsum", bufs=4, space="PSUM") as opsum,
        ):
            pending_store = [None]

            def flush_store():
                if pending_store[0] is not None:
                    osb_prev, pair_prev = pending_store[0]
                    nc.sync.dma_start(out=out_d[pair_prev], in_=osb_prev)
                    pending_store[0] = None

            for pair in range(ppc):
                es_t = small.tile([P, 1], F32, tag="esink")
                nc.sync.dma_start(
                    out=es_t, in_=es_d[pair, :].rearrange("(p o) -> p o", o=1)
                )
                # whole-pair loads (large contiguous DMA runs)
                qtb = loads.tile([P, s_len], BF16, tag="qtb")
                nc.sync.dma_start(out=qtb, in_=qt_d[pair])
                ktb = loads.tile([P, s_len], BF16, tag="ktb")
                nc.sync.dma_start(out=ktb, in_=kt_d[pair])
                vb = loads.tile([P, nch, T, D + 1], BF16, tag="vb")
                nc.sync.dma_start(out=vb, in_=v_d[pair])
                flush_store()

                osb = opool.tile([P, nch, T, D], F32, tag="osb")
                for ch in range(nch):
                    s0 = ch * C

                    # Phase 1: scores + exp per key-tile
                    pt_all = ppool.tile([P, T, C], BF16, tag="pt")
                    for kt in range(T):
                        c0 = kt * P
                        st = spsum.tile([P, C], F32, tag="st")
                        # split matmuls at the PSUM bank boundary (col 512)
                        if c0 < 512:
                            spans = [(c0, 512), (512, C)]
                        else:
                            spans = [(c0, C)]
                        for a, b_ in spans:
                            nc.tensor.matmul(
                                st[:, a:b_],
                                ktb[:, s0 + c0:s0 + c0 + P],
                                qtb[:, s0 + a:s0 + b_],
                                start=True,
                                stop=True,
                            )
                        nc.scalar.activation(
                            pt_all[:, kt, c0:C],
                            st[:, c0:C],
                            mybir.ActivationFunctionType.Exp,
                            scale=SCALE,
                        )
                        # zero the strictly-upper (k > q) part of the diag block
                        nc.gpsimd.affine_select(
                            out=pt_all[:, kt, c0:c0 + P],
                            in_=pt_all[:, kt, c0:c0 + P],
                            compare_op=mybir.AluOpType.is_ge,
                            fill=0.0,
                            base=0,
                            channel_multiplier=-1,
                            pattern=[[1, P]],
                        )

                    # Phase 2: PV accumulation + normalize, two query-tiles per
                    # PSUM bank (2 x 129 floats = 1032B fits one 2KB bank and
                    # neither matmul output crosses the bank boundary).
                    for j in range(T // 2):
                        oacc = opsum.tile([P, 2, D + 1], F32, tag="oacc")
                        for qq in range(2):
                            qt = 2 * j + qq
                            for kt in range(qt + 1):
                                nc.tensor.matmul(
                                    oacc[:, qq, :],
                                    pt_all[:, kt, qt * P:(qt + 1) * P],
                                    vb[:, ch, kt, :],
                                    start=(kt == 0),
                                    stop=(kt == qt),
                                )
                        den = small.tile([P, 2], F32, tag="den")
                        nc.vector.tensor_scalar_add(den, oacc[:, :, D], es_t)
                        rec = small.tile([P, 2], F32, tag="rec")
                        nc.vector.reciprocal(rec, den)
                        rec_b = bass.AP(
                            tensor=rec.tensor,
                            offset=rec.offset,
                            ap=[rec.ap[0], [1, 2], [0, D]],
                        )
                        nc.vector.tensor_tensor(
                            osb[:, ch, 2 * j:2 * j + 2, :],
                            oacc[:, :, 0:D],
                            rec_b,
                            mybir.AluOpType.mult,
                        )

                pending_store[0] = (osb, pair)
            flush_store()

    nc.compile()
    return nc


_PROGRAM = None


def _get_program():
    global _PROGRAM
    if _PROGRAM is None:
        _PROGRAM = _build_program()
    return _PROGRAM


def _prep_in_maps(q, k, v, sinks):
    # [B,S,H,D] -> [B*H, S, D]
    qp = np.ascontiguousarray(q.transpose(0, 2, 1, 3)).reshape(PAIRS, S, D)
    kp = np.ascontiguousarray(k.transpose(0, 2, 1, 3)).reshape(PAIRS, S, D)
    vp = np.ascontiguousarray(v.transpose(0, 2, 1, 3)).reshape(PAIRS, S, D)
    # Q, K additionally transposed to [pairs, D, S] bf16 (matmul layout/dtype)
    qT = np.ascontiguousarray(qp.transpose(0, 2, 1)).astype(ml_dtypes.bfloat16)
    kT = np.ascontiguousarray(kp.transpose(0, 2, 1)).astype(ml_dtypes.bfloat16)
    # V: bf16, partition-major [pairs, P, nch, T, D+1] with a ones column
    vaug = np.empty((PAIRS, NCH, T, P, D + 1), dtype=ml_dtypes.bfloat16)
    vaug[..., :D] = vp.reshape(PAIRS, NCH, T, P, D).astype(ml_dtypes.bfloat16)
    vaug[..., D] = np.asarray(1.0, ml_dtypes.bfloat16)
    vaug = np.ascontiguousarray(vaug.transpose(0, 3, 1, 2, 4))
    es_pairs = np.tile(np.exp(sinks), B)  # es_pairs[i] = exp(sinks[i % H])
    esb = np.repeat(es_pairs[:, None], P, axis=1).astype(np.float32)

    in_maps = []
    for c in range(N_CORES):
        sl = slice(c * PPC, (c + 1) * PPC)
        in_maps.append(
            {"qt": qT[sl], "kt": kT[sl], "v": vaug[sl], "esink": esb[sl]}
        )
    return in_maps


def kernel(q, k, v, sinks, chunk_size):
    assert int(chunk_size) == C
    q = np.asarray(q, dtype=np.float32)
    k = np.asarray(k, dtype=np.float32)
    v = np.asarray(v, dtype=np.float32)
    sinks = np.asarray(sinks, dtype=np.float32)
    assert q.shape == (B, S, H, D)

    in_maps = _prep_in_maps(q, k, v, sinks)
    nc = _get_program()
    res = run_bass_kernel_spmd(nc, in_maps, core_ids=list(range(N_CORES)))

    outp = np.concatenate([res.results[c]["out"] for c in range(N_CORES)], axis=0)
    # [pairs, p, chunk, t, d] -> [pairs, s, d] (s = chunk*C + t*P + p)
    outp = outp.transpose(0, 2, 3, 1, 4).reshape(PAIRS, S, D)
    out = outp.reshape(B, H, S, D).transpose(0, 2, 1, 3)
    return np.ascontiguousarray(out)


# revision 22
# speedup vs baseline: 1.0225x; 1.0225x over previous
"""Chunked-causal attention with sinks on 8 TRN2 NeuronCores.

Sharding: the 64 (batch, head) pairs are split 8-per-core (data parallel on
B, tensor parallel on H). Each core runs the same Bass program over its 8
pairs x 4 chunks of 1024 tokens.

The per-core shard layout is chosen for DMA/TensorE efficiency:
  - Q, K arrive pre-transposed as bf16 [pairs, D, S]: the score matmul
    contracts over D, which must sit on SBUF partitions, and bf16 is the
    matmul compute dtype either way (the host conversion is numerically
    identical to an on-device cast). Per-partition rows are contiguous.
  - V arrives as bf16 [pairs, P, nch, T, D+1] (s = t*P + p within a chunk),
    with a ones column appended: partition-major so each partition's slice
    is one contiguous DRAM run, and the ones column makes the PV matmul
    emit the softmax denominator as output column D.
  - The output is stored partition-major [pairs, nch, P, T, D] and
    un-permuted on the host.

Per (pair, chunk) the kernel computes, entirely on-chip:
  S_T[k, q] = K @ Q^T          (TensorE, bf16; scores transposed so that the
                                PV matmul can consume exp(S_T) directly)
  P_T       = exp(S_T / sqrt(D))  (ScalarE; no max-subtraction -- softmax is
                                shift-invariant and scores here are O(5), so
                                fp32 exp is exact enough; masked entries are
                                zeroed after the exp)
  O[q, :]   = P_T^T @ [V | 1]  (TensorE; the ones column yields the softmax
                                denominator in column D of the same matmul)
  out       = O[:, :D] / (O[:, D] + exp(sink))
"""

import ml_dtypes
import numpy as np

import concourse.bacc as bacc
import concourse.bass as bass
import concourse.mybir as mybir
import concourse.tile as tile
from concourse.bass_utils import run_bass_kernel_spmd

N_CORES = 8
B, S, H, D = 4, 4096, 16, 128
C = 1024                # chunk size
NCH = S // C            # chunks per sequence
PAIRS = B * H           # 64 (batch, head) pairs
PPC = PAIRS // N_CORES  # pairs per core
P = 128                 # SBUF partitions
T = C // P              # 128-row tiles per chunk
SCALE = 1.0 / float(np.sqrt(D))

F32 = mybir.dt.float32
BF16 = mybir.dt.bfloat16


def _build_program(ppc=PPC, nch=NCH):
    s_len = nch * C
    nc = bacc.Bacc("TRN2", target_bir_lowering=False, debug=False)
    qt_d = nc.dram_tensor("qt", [ppc, D, s_len], BF16, kind="ExternalInput")
    kt_d = nc.dram_tensor("kt", [ppc, D, s_len], BF16, kind="ExternalInput")
    v_d = nc.dram_tensor("v", [ppc, P, nch, T, D + 1], BF16, kind="ExternalInput")
    es_d = nc.dram_tensor("esink", [ppc, P], F32, kind="ExternalInput")
    out_d = nc.dram_tensor("out", [ppc, P, nch, T, D], F32, kind="ExternalOutput")

    with tile.TileContext(nc) as tc:
        with (
            tc.tile_pool(name="loads", bufs=3) as loads,
            tc.tile_pool(name="ptile", bufs=2) as ppool,
            tc.tile_pool(name="outs", bufs=2) as opool,
            tc.tile_pool(name="small", bufs=4) as small,
            tc.tile_pool(name="spsum", bufs=2, space="PSUM") as spsum,
            tc.tile_pool(name="opsum", bufs=4, space="PSUM") as opsum,
        ):
            pending_store = [None]

            def flush_store():
                if pending_store[0] is not None:
                    osb_prev, pair_prev, ch_prev = pending_store[0]
                    nc.sync.dma_start(
                        out=out_d[pair_prev, :, ch_prev], in_=osb_prev
                    )
                    pending_store[0] = None

            for pair in range(ppc):
                es_t = small.tile([P, 1], F32, tag="esink")
                nc.sync.dma_start(
                    out=es_t, in_=es_d[pair, :].rearrange("(p o) -> p o", o=1)
                )
                for ch in range(nch):
                    s0 = ch * C
                    qtb = loads.tile([P, C], BF16, tag="qtb")
                    nc.sync.dma_start(out=qtb, in_=qt_d[pair, :, s0:s0 + C])
                    ktb = loads.tile([P, C], BF16, tag="ktb")
                    nc.sync.dma_start(out=ktb, in_=kt_d[pair, :, s0:s0 + C])
                    vb = loads.tile([P, T, D + 1], BF16, tag="vb")
                    nc.sync.dma_start(out=vb, in_=v_d[pair, :, ch])
                    flush_store()

                    # Phase 1: scores + exp per key-tile
                    pt_all = ppool.tile([P, T, C], BF16, tag="pt")
                    for kt in range(T):
                        c0 = kt * P
                        st = spsum.tile([P, C], F32, tag="st")
                        # split matmuls at the PSUM bank boundary (col 512)
                        if c0 < 512:
                            spans = [(c0, 512), (512, C)]
                        else:
                            spans = [(c0, C)]
                        for a, b_ in spans:
                            nc.tensor.matmul(
                                st[:, a:b_],
                                ktb[:, c0:c0 + P],
                                qtb[:, a:b_],
                                start=True,
                                stop=True,
                            )
                        nc.scalar.activation(
                            pt_all[:, kt, c0:C],
                            st[:, c0:C],
                            mybir.ActivationFunctionType.Exp,
                            scale=SCALE,
                        )
                        # zero the strictly-upper (k > q) part of the diag block
                        nc.gpsimd.affine_select(
                            out=pt_all[:, kt, c0:c0 + P],
                            in_=pt_all[:, kt, c0:c0 + P],
                            compare_op=mybir.AluOpType.is_ge,
                            fill=0.0,
                            base=0,
                            channel_multiplier=-1,
                            pattern=[[1, P]],
                        )

                    # Phase 2: PV accumulation + normalize, two query-tiles per
                    # PSUM bank (2 x 129 floats = 1032B fits one 2KB bank and
                    # neither matmul output crosses the bank boundary).
                    osb = opool.tile([P, T, D], F32, tag="osb")
                    for j in range(T // 2):
                        oacc = opsum.tile([P, 2, D + 1], F32, tag="oacc")
                        for qq in range(2):
                            qt = 2 * j + qq
                            for kt in range(qt + 1):
                                nc.tensor.matmul(
                                    oacc[:, qq, :],
                                    pt_all[:, kt, qt * P:(qt + 1) * P],
                                    vb[:, kt, :],
                                    start=(kt == 0),
                                    stop=(kt == qt),
                                )
                        den = small.tile([P, 2], F32, tag="den")
                        nc.vector.tensor_scalar_add(den, oacc[:, :, D], es_t)
                        rec = small.tile([P, 2], F32, tag="rec")
                        nc.vector.reciprocal(rec, den)
                        rec_b = bass.AP(
                            tensor=rec.tensor,
                            offset=rec.offset,
                            ap=[rec.ap[0], [1, 2], [0, D]],
                        )
                        nc.vector.tensor_tensor(
                            osb[:, 2 * j:2 * j + 2, :],
                            oacc[:, :, 0:D],
                            rec_b,
                            mybir.AluOpType.mult,
                        )

                    pending_store[0] = (osb, pair, ch)
            flush_store()

    nc.compile()
    return nc


_PROGRAM = None


def _get_program():
    global _PROGRAM
    if _PROGRAM is None:
        _PROGRAM = _build_program()
    return _PROGRAM


def _prep_in_maps(q, k, v, sinks):
    # [B,S,H,D] -> [B*H, S, D]
    qp = np.ascontiguousarray(q.transpose(0, 2, 1, 3)).reshape(PAIRS, S, D)
    kp = np.ascontiguousarray(k.transpose(0, 2, 1, 3)).reshape(PAIRS, S, D)
    vp = np.ascontiguousarray(v.transpose(0, 2, 1, 3)).reshape(PAIRS, S, D)
    # Q, K additionally transposed to [pairs, D, S] bf16 (matmul layout/dtype)
    qT = np.ascontiguousarray(qp.transpose(0, 2, 1)).astype(ml_dtypes.bfloat16)
    kT = np.ascontiguousarray(kp.transpose(0, 2, 1)).astype(ml_dtypes.bfloat16)
    # V: bf16, partition-major [pairs, P, nch, T, D+1] with a ones column
    vaug = np.empty((PAIRS, NCH, T, P, D + 1), dtype=ml_dtypes.bfloat16)
    vaug[..., :D] = vp.reshape(PAIRS, NCH, T, P, D).astype(ml_dtypes.bfloat16)
    vaug[..., D] = np.asarray(1.0, ml_dtypes.bfloat16)
    vaug = np.ascontiguousarray(vaug.transpose(0, 3, 1, 2, 4))
    es_pairs = np.tile(np.exp(sinks), B)  # es_pairs[i] = exp(sinks[i % H])
    esb = np.repeat(es_pairs[:, None], P, axis=1).astype(np.float32)

    in_maps = []
    for c in range(N_CORES):
        sl = slice(c * PPC, (c + 1) * PPC)
        in_maps.append(
            {"qt": qT[sl], "kt": kT[sl], "v": vaug[sl], "esink": esb[sl]}
        )
    return in_maps


def kernel(q, k, v, sinks, chunk_size):
    assert int(chunk_size) == C
    q = np.asarray(q, dtype=np.float32)
    k = np.asarray(k, dtype=np.float32)
    v = np.asarray(v, dtype=np.float32)
    sinks = np.asarray(sinks, dtype=np.float32)
    assert q.shape == (B, S, H, D)

    in_maps = _prep_in_maps(q, k, v, sinks)
    nc = _get_program()
    res = run_bass_kernel_spmd(nc, in_maps, core_ids=list(range(N_CORES)))

    outp = np.concatenate([res.results[c]["out"] for c in range(N_CORES)], axis=0)
    # [pairs, p, chunk, t, d] -> [pairs, s, d] (s = chunk*C + t*P + p)
    outp = outp.transpose(0, 2, 3, 1, 4).reshape(PAIRS, S, D)
    out = outp.reshape(B, H, S, D).transpose(0, 2, 1, 3)
    return np.ascontiguousarray(out)
